# revision 2
# baseline (speedup 1.0000x reference)
"""Multi-head attention (B=2, S=2048, D=1024, H=16) on 8 Trainium2 NeuronCores.

Sharding strategy (sequence-parallel, chosen over the hinted head-TP +
all-reduce because it needs only two small AllGathers instead of a 16.8MB
AllReduce):
  - The B*S = 4096 token rows are split 512/core. Cores 0-3 own batch 0,
    cores 4-7 own batch 1.
  - Each core computes Q/K/V projections for its own 512 tokens, then the
    K^T and V projection results are AllGathered within each 4-core batch
    group (2.1 MB/core -> 8.4 MB), overlapping with the Q projection.
  - Each core then runs full 16-head attention for its 512 query rows and
    the row-slice of the output projection. The full output is a pure
    host-side concatenation - no reduction.

Layout strategy: scores are computed transposed (K @ Q^T per head) so that
  - the exp-scores [k, q] feed the P*V matmul directly as the moving operand,
  - the softmax denominator is produced for free by a ones-column appended to
    V inside the same accumulation ([V | 1]^T @ expT),
  - the P*V output CT[d, q] is directly the stationary operand of the output
    projection. No transposes anywhere.
Normalization (divide by denominator, a per-q scale under a [d, q] layout) is
applied by broadcasting 1/denom across partitions with a K=1 ones matmul and
one vector multiply.

dtype: float32r (TF32-like, ~1.6e-4 per matmul) for all matmuls, fp32 psum
accumulation and softmax. The max-subtraction in the reference softmax is an
exact no-op mathematically and is skipped (scores ~ N(0,1), no overflow);
the reference's +1e-9 on the denominator is below f32r resolution (denom>=1).
"""

import sys

if "/opt/trn_rl_repo" not in sys.path:
    sys.path.insert(0, "/opt/trn_rl_repo")

import numpy as np

B, S, D = 2, 2048, 1024
H, DK = 16, 64
N_CORES = 8
DT = B * S // N_CORES          # 512 tokens per core
NB = 4                         # cores per batch group
KB = NB * DT                   # 2048 keys per batch group
GROUPS = [[0, 1, 2, 3], [4, 5, 6, 7]]

_CACHE = {}


def _build():
    import concourse.bass as bass
    import concourse.bacc as bacc
    import concourse.mybir as mybir
    import concourse.tile as tile
    from contextlib import ExitStack

    f32 = mybir.dt.float32
    f32r = mybir.dt.float32r
    EXP = mybir.ActivationFunctionType.Exp

    nc = bacc.Bacc("TRN2", target_bir_lowering=False, debug=False,
                   num_devices=N_CORES)

    # ---- I/O ----
    qT = nc.dram_tensor("qT", [D, DT], f32r, kind="ExternalInput")
    kT = nc.dram_tensor("kT", [D, DT], f32r, kind="ExternalInput")
    vT = nc.dram_tensor("vT", [D, DT], f32r, kind="ExternalInput")
    wqT = nc.dram_tensor("wqT", [D, D], f32r, kind="ExternalInput")
    wkT = nc.dram_tensor("wkT", [D, D], f32r, kind="ExternalInput")
    wvT = nc.dram_tensor("wvT", [D, D], f32r, kind="ExternalInput")
    woT = nc.dram_tensor("woT", [D, D], f32r, kind="ExternalInput")
    bq = nc.dram_tensor("bq", [D], f32, kind="ExternalInput")
    bk = nc.dram_tensor("bk", [D], f32, kind="ExternalInput")
    bv = nc.dram_tensor("bv", [D], f32r, kind="ExternalInput")
    bo = nc.dram_tensor("bo", [D], f32r, kind="ExternalInput")
    onesin = nc.dram_tensor("onesin", [128, 128], f32r, kind="ExternalInput")
    out = nc.dram_tensor("out", [DT, D], f32, kind="ExternalOutput")

    with tile.TileContext(nc) as tc, ExitStack() as top:
        # ---- long-lived tiles ----
        const = top.enter_context(tc.tile_pool(name="const", bufs=1))
        ones_sb = const.tile([128, 128], f32r, tag="ones")
        nc.sync.dma_start(ones_sb[:], onesin.ap())
        bq_sb = const.tile([128, 8], f32, tag="bq")
        nc.sync.dma_start(bq_sb[:], bq.ap().rearrange("(a p) -> p a", p=128))
        bk_sb = const.tile([128, 8], f32, tag="bk")
        nc.sync.dma_start(bk_sb[:], bk.ap().rearrange("(a p) -> p a", p=128))
        bv_sb = const.tile([1, D], f32r, tag="bv")
        nc.sync.dma_start(bv_sb[:], bv.ap().rearrange("(a d) -> a d", a=1))
        bo_sb = const.tile([1, D], f32r, tag="bo")
        nc.sync.dma_start(bo_sb[:], bo.ap().rearrange("(a d) -> a d", a=1))

        qt_pool = top.enter_context(tc.tile_pool(name="qt", bufs=1))
        qt_sb = [qt_pool.tile([128, DT], f32r, tag=f"qt{s}", name=f"qt{s}")
                 for s in range(8)]
        ct_pool = top.enter_context(tc.tile_pool(name="ct", bufs=1))
        ct_sb = [ct_pool.tile([128, DT], f32r, tag=f"ct{t}", name=f"ct{t}")
                 for t in range(8)]

        dram = top.enter_context(tc.tile_pool(name="dram", bufs=1, space="DRAM"))
        # NOTE: Shared-output AG needs >4-core groups; Local is required here.
        ktp_in = dram.tile([D, DT], f32r, tag="ktp_in", name="ktp_in")
        ktp_out = dram.tile([NB * D, DT], f32r, tag="ktp_out", name="ktp_out")
        vtp_in = dram.tile([DT, D], f32r, tag="vtp_in", name="vtp_in")
        vtp_out = dram.tile([KB, D], f32r, tag="vtp_out", name="vtp_out")

        # ---- phase B: K/V projections + AllGathers ----
        with ExitStack() as ph:
            inp = ph.enter_context(tc.tile_pool(name="inp", bufs=1))
            wpool = ph.enter_context(tc.tile_pool(name="wpool", bufs=6))
            wvpool = ph.enter_context(tc.tile_pool(name="wvpool", bufs=16))
            stg = ph.enter_context(tc.tile_pool(name="stg", bufs=4))
            psp = ph.enter_context(tc.tile_pool(name="psp", bufs=3, space="PSUM"))

            kin = [inp.tile([128, DT], f32r, tag=f"kin{i}", name=f"kin{i}")
                   for i in range(8)]
            for i in range(8):
                nc.sync.dma_start(kin[i][:], kT.ap()[i * 128:(i + 1) * 128, :])

            # K^T projection: KT[dout, tok] = sum_din Wk[dout,din] * k[tok,din]
            for s in range(8):
                pp = psp.tile([128, DT], f32, tag="pp", name="pp")
                for i in range(8):
                    w = wpool.tile([128, 128], f32r, tag="w", name="w")
                    nc.sync.dma_start(
                        w[:], wkT.ap()[i * 128:(i + 1) * 128, s * 128:(s + 1) * 128])
                    nc.tensor.matmul(pp[:], w[:], kin[i][:],
                                     start=(i == 0), stop=(i == 7))
                st = stg.tile([128, DT], f32r, tag="st", name="st")
                nc.scalar.add(st[:], pp[:], bk_sb[:, s:s + 1])
                nc.sync.dma_start(ktp_in[s * 128:(s + 1) * 128, :], st[:])

            nc.gpsimd.collective_compute(
                "AllGather", mybir.AluOpType.bypass, replica_groups=GROUPS,
                ins=[ktp_in.opt()], outs=[ktp_out.opt()])

            vin = [inp.tile([128, DT], f32r, tag=f"vin{i}", name=f"vin{i}")
                   for i in range(8)]
            for i in range(8):
                nc.sync.dma_start(vin[i][:], vT.ap()[i * 128:(i + 1) * 128, :])

            # V projection (natural layout): V[tok, dout]
            for dh in range(2):
                wv = []
                for i in range(8):
                    wt = wvpool.tile([128, 512], f32r, tag="wv", name="wv")
                    nc.sync.dma_start(
                        wt[:], wvT.ap()[i * 128:(i + 1) * 128, dh * 512:(dh + 1) * 512])
                    wv.append(wt)
                for ks in range(4):
                    pp = psp.tile([128, 512], f32, tag="pp", name="pp")
                    for i in range(8):
                        nc.tensor.matmul(pp[:], vin[i][:, ks * 128:(ks + 1) * 128],
                                         wv[i][:], start=(i == 0), stop=False)
                    nc.tensor.matmul(pp[:], ones_sb[0:1, :],
                                     bv_sb[0:1, dh * 512:(dh + 1) * 512],
                                     start=False, stop=True)
                    st = stg.tile([128, 512], f32r, tag="st", name="st")
                    nc.scalar.copy(st[:], pp[:])
                    nc.sync.dma_start(
                        vtp_in[ks * 128:(ks + 1) * 128, dh * 512:(dh + 1) * 512], st[:])

            nc.gpsimd.collective_compute(
                "AllGather", mybir.AluOpType.bypass, replica_groups=GROUPS,
                ins=[vtp_in.opt()], outs=[vtp_out.opt()])

            # Q^T projection (overlaps the collectives)
            qin = [inp.tile([128, DT], f32r, tag=f"qin{i}", name=f"qin{i}")
                   for i in range(8)]
            for i in range(8):
                nc.sync.dma_start(qin[i][:], qT.ap()[i * 128:(i + 1) * 128, :])
            for s in range(8):
                pp = psp.tile([128, DT], f32, tag="pp", name="pp")
                for i in range(8):
                    w = wpool.tile([128, 128], f32r, tag="w", name="w")
                    nc.sync.dma_start(
                        w[:], wqT.ap()[i * 128:(i + 1) * 128, s * 128:(s + 1) * 128])
                    nc.tensor.matmul(pp[:], w[:], qin[i][:],
                                     start=(i == 0), stop=(i == 7))
                nc.scalar.add(qt_sb[s][:], pp[:], bq_sb[:, s:s + 1])

        # ---- phase D: attention ----
        NJ = KB // 128            # 16 key tiles of 128
        with ExitStack() as ph:
            vpl_pool = ph.enter_context(tc.tile_pool(name="vpl", bufs=1))
            ktf_pool = ph.enter_context(tc.tile_pool(name="ktf", bufs=8))
            ex_pool = ph.enter_context(tc.tile_pool(name="ex", bufs=3))
            rc_pool = ph.enter_context(tc.tile_pool(name="rc", bufs=2))
            bc_pool = ph.enter_context(tc.tile_pool(name="bc", bufs=2))
            ps_s = ph.enter_context(tc.tile_pool(name="ps_s", bufs=2, space="PSUM"))
            ps_ct = ph.enter_context(tc.tile_pool(name="ps_ct", bufs=2, space="PSUM"))
            ps_b = ph.enter_context(tc.tile_pool(name="ps_b", bufs=1, space="PSUM"))

            # V with interleaved ones columns: [128, 16*(64+1)]
            vp = []
            for j in range(NJ):
                vpt = vpl_pool.tile([128, H * (DK + 1)], f32r,
                                    tag=f"vp{j}", name=f"vp{j}")
                dst = vpt[:].rearrange("p (h d) -> p h d", h=H)[:, :, 0:DK]
                src = vtp_out[j * 128:(j + 1) * 128, :].rearrange(
                    "p (h d) -> p h d", h=H)
                nc.sync.dma_start(dst, src)
                oc = vpt[:].rearrange("p (h d) -> p h d", h=H)[:, :, DK:DK + 1]
                nc.sync.dma_start(
                    oc, onesin.ap().rearrange("p (h d) -> p h d", h=H)[:, :, 0:1])
                vp.append(vpt)

            for t in range(8):                      # head pairs
                ktf = []
                for r in range(NB):
                    kt_t = ktf_pool.tile([128, DT], f32r, tag="ktf", name="ktf")
                    nc.sync.dma_start(
                        kt_t[:], ktp_out[r * D + t * 128: r * D + (t + 1) * 128, :])
                    ktf.append(kt_t)
                for hh in range(2):
                    h = 2 * t + hh
                    pct = ps_ct.tile([65, DT], f32, tag="pct", name="pct")
                    for g in range(NJ // 2):        # pairs of key tiles
                        pss = ps_s.tile([128, 2 * DT], f32, tag="pss", name="pss")
                        ex = ex_pool.tile([128, 2 * DT], f32r, tag="ex", name="ex")
                        for u in range(2):
                            j = 2 * g + u
                            r, jl = j // NB, j % NB
                            nc.tensor.matmul(
                                pss[:, u * DT:(u + 1) * DT],
                                ktf[r][hh * 64:hh * 64 + 64, jl * 128:(jl + 1) * 128],
                                qt_sb[t][hh * 64:hh * 64 + 64, :],
                                start=True, stop=True)
                        nc.scalar.activation(ex[:], pss[:], EXP, scale=0.125)
                        for u in range(2):
                            j = 2 * g + u
                            nc.tensor.matmul(
                                pct[:], vp[j][:, h * 65:(h + 1) * 65],
                                ex[:, u * DT:(u + 1) * DT],
                                start=(j == 0), stop=(j == NJ - 1))
                    rc = rc_pool.tile([1, DT], f32r, tag="rc", name="rc")
                    with nc.allow_low_precision(reason="f32r softmax recip"):
                        nc.vector.reciprocal(rc[:], pct[64:65, :])
                    pb = ps_b.tile([128, DT], f32, tag="pb", name="pb")
                    nc.tensor.matmul(pb[:], ones_sb[0:1, :], rc[:],
                                     start=True, stop=True)
                    bcst = bc_pool.tile([128, DT], f32, tag="bc", name="bc")
                    nc.vector.tensor_copy(bcst[:], pb[:])
                    with nc.allow_low_precision(reason="f32r ctx normalize"):
                        nc.vector.tensor_mul(
                            ct_sb[t][hh * 64:hh * 64 + 64, :],
                            pct[0:64, :], bcst[0:64, :])

        # ---- phase E: output projection ----
        with ExitStack() as ph:
            wo_pool = ph.enter_context(tc.tile_pool(name="wo", bufs=16))
            ob_pool = ph.enter_context(tc.tile_pool(name="ob", bufs=3))
            ps_o = ph.enter_context(tc.tile_pool(name="ps_o", bufs=3, space="PSUM"))
            for dh in range(2):
                wo = []
                for tt in range(8):
                    wt = wo_pool.tile([128, 512], f32r, tag="wo", name="wo")
                    nc.sync.dma_start(
                        wt[:], woT.ap()[tt * 128:(tt + 1) * 128,
                                        dh * 512:(dh + 1) * 512])
                    wo.append(wt)
                for qs in range(4):
                    po = ps_o.tile([128, 512], f32, tag="po", name="po")
                    for tt in range(8):
                        nc.tensor.matmul(po[:],
                                         ct_sb[tt][:, qs * 128:(qs + 1) * 128],
                                         wo[tt][:], start=(tt == 0), stop=False)
                    nc.tensor.matmul(po[:], ones_sb[0:1, :],
                                     bo_sb[0:1, dh * 512:(dh + 1) * 512],
                                     start=False, stop=True)
                    ob = ob_pool.tile([128, 512], f32, tag="ob", name="ob")
                    nc.scalar.copy(ob[:], po[:])
                    nc.sync.dma_start(
                        out.ap()[qs * 128:(qs + 1) * 128,
                                 dh * 512:(dh + 1) * 512], ob[:])

    nc.compile()
    return nc


def _get_nc():
    if "nc" not in _CACHE:
        _CACHE["nc"] = _build()
    return _CACHE["nc"]


def _prep_in_maps(q, k, v, Wq, bq, Wk, bk, Wv, bv, Wo, bo):
    f = lambda a: np.ascontiguousarray(np.asarray(a, dtype=np.float32))
    qT = f(np.asarray(q, dtype=np.float32).reshape(B * S, D).T)
    kT = f(np.asarray(k, dtype=np.float32).reshape(B * S, D).T)
    vT = f(np.asarray(v, dtype=np.float32).reshape(B * S, D).T)
    shared = {
        "wqT": f(np.asarray(Wq).T), "wkT": f(np.asarray(Wk).T),
        "wvT": f(np.asarray(Wv).T), "woT": f(np.asarray(Wo).T),
        "bq": f(bq), "bk": f(bk), "bv": f(bv), "bo": f(bo),
        "onesin": np.ones((128, 128), np.float32),
    }
    in_maps = []
    for c in range(N_CORES):
        sl = slice(c * DT, (c + 1) * DT)
        in_maps.append({
            "qT": np.ascontiguousarray(qT[:, sl]),
            "kT": np.ascontiguousarray(kT[:, sl]),
            "vT": np.ascontiguousarray(vT[:, sl]),
            **shared,
        })
    return in_maps


def _run(in_maps, trace=False, **kw):
    from concourse.bass_utils import run_bass_kernel_spmd
    nc = _get_nc()
    res = run_bass_kernel_spmd(nc, in_maps, core_ids=list(range(N_CORES)),
                               trace=trace, **kw)
    full = np.concatenate([res.results[c]["out"] for c in range(N_CORES)],
                          axis=0).reshape(B, S, D)
    return full, res


def kernel(q, k, v, Wq, bq, Wk, bk, Wv, bv, Wo, bo):
    in_maps = _prep_in_maps(q, k, v, Wq, bq, Wk, bk, Wv, bv, Wo, bo)
    full, _ = _run(in_maps, trace=False)
    return full


# revision 5
# speedup vs baseline: 1.2053x; 1.2053x over previous
"""Multi-head attention (B=2, S=2048, D=1024, H=16) on 8 Trainium2 NeuronCores.

Sharding (sequence-parallel; chosen over the hinted head-TP + all-reduce
because it needs only one small AllGather instead of a 16.8MB AllReduce):
  - B*S = 4096 token rows split 512/core; cores 0-3 own batch 0, 4-7 batch 1.
  - Each core: Q^T projection for its tokens; K^T projection for its tokens
    then AllGather within the 4-core batch group (2.1MB/core); V projection
    computed fully per-core (cheaper than a second AllGather and it fills the
    PE while the K AllGather is in flight), written straight into SBUF.
  - Full 16-head attention for the core's 512 query rows, then the row-slice
    of the output projection. Host output assembly is pure concatenation.

Layout: scores computed transposed (K @ Q^T per head) so exp-scores feed the
P*V matmul as the moving operand, the softmax denominator comes free via a
ones-column interleaved into V ([V_h | 1] stationary), and the P*V output
CT[d, q] is directly the stationary operand of the output projection. The
per-q normalization is broadcast across partitions with a K=1 ones matmul.

PE scheduling: QK matmuls for the two heads of a pair alternate between
array row-groups 0-63 / 64-127 (KT tiles hold head pairs on the partition
axis), which lets consecutive dk=64 matmuls overlap in disjoint array halves
(~139ns vs 427ns measured). P*V (K=128) runs at full rate.

dtype: float32r matmuls (~1.6e-4/matmul), fp32 psum + softmax. Reference's
max-subtraction is an exact no-op (scores ~N(0,1)) and is skipped; its +1e-9
on the denominator is below f32r resolution (denom >= 1).
"""

import sys

if "/opt/trn_rl_repo" not in sys.path:
    sys.path.insert(0, "/opt/trn_rl_repo")

import numpy as np

B, S, D = 2, 2048, 1024
H, DK = 16, 64
N_CORES = 8
DT = B * S // N_CORES          # 512 tokens per core
NB = 4                         # cores per batch group
KB = NB * DT                   # 2048 keys per batch group
NJ = KB // 128                 # 16 key tiles
GROUPS = [[0, 1, 2, 3], [4, 5, 6, 7]]

_CACHE = {}


def _build():
    import concourse.bass as bass
    import concourse.bacc as bacc
    import concourse.mybir as mybir
    import concourse.tile as tile
    from contextlib import ExitStack

    f32 = mybir.dt.float32
    f32r = mybir.dt.float32r
    EXP = mybir.ActivationFunctionType.Exp

    nc = bacc.Bacc("TRN2", target_bir_lowering=False, debug=False,
                   num_devices=N_CORES)

    # ---- I/O ----
    qT = nc.dram_tensor("qT", [D, DT], f32r, kind="ExternalInput")
    kT = nc.dram_tensor("kT", [D, DT], f32r, kind="ExternalInput")
    vT = nc.dram_tensor("vT", [D, KB], f32r, kind="ExternalInput")  # full batch
    wqT = nc.dram_tensor("wqT", [D, D], f32r, kind="ExternalInput")
    wkT = nc.dram_tensor("wkT", [D, D], f32r, kind="ExternalInput")
    wvT = nc.dram_tensor("wvT", [D, D], f32r, kind="ExternalInput")
    woT = nc.dram_tensor("woT", [D, D], f32r, kind="ExternalInput")
    bq = nc.dram_tensor("bq", [D], f32, kind="ExternalInput")
    bk = nc.dram_tensor("bk", [D], f32, kind="ExternalInput")
    bv = nc.dram_tensor("bv", [D], f32r, kind="ExternalInput")
    bo = nc.dram_tensor("bo", [D], f32r, kind="ExternalInput")
    onesin = nc.dram_tensor("onesin", [128, 128], f32r, kind="ExternalInput")
    out = nc.dram_tensor("out", [DT, D], f32, kind="ExternalOutput")

    with tile.TileContext(nc) as tc, ExitStack() as top:
        # ---- long-lived tiles ----
        const = top.enter_context(tc.tile_pool(name="const", bufs=1))
        ones_sb = const.tile([128, 128], f32r, tag="ones")
        nc.sync.dma_start(ones_sb[:], onesin.ap())
        bq_sb = const.tile([128, 8], f32, tag="bq")
        nc.sync.dma_start(bq_sb[:], bq.ap().rearrange("(a p) -> p a", p=128))
        bk_sb = const.tile([128, 8], f32, tag="bk")
        nc.sync.dma_start(bk_sb[:], bk.ap().rearrange("(a p) -> p a", p=128))
        bv_sb = const.tile([1, D], f32r, tag="bv")
        nc.sync.dma_start(bv_sb[:], bv.ap().rearrange("(a d) -> a d", a=1))
        bo_sb = const.tile([1, D], f32r, tag="bo")
        nc.sync.dma_start(bo_sb[:], bo.ap().rearrange("(a d) -> a d", a=1))

        qt_pool = top.enter_context(tc.tile_pool(name="qt", bufs=1))
        qt_sb = [qt_pool.tile([128, DT], f32r, tag=f"qt{s}", name=f"qt{s}")
                 for s in range(8)]
        ct_pool = top.enter_context(tc.tile_pool(name="ct", bufs=1))
        ct_sb = [ct_pool.tile([128, DT], f32r, tag=f"ct{t}", name=f"ct{t}")
                 for t in range(8)]
        # V with interleaved ones columns: [128, 16*(64+1)] per key tile
        vpl_pool = top.enter_context(tc.tile_pool(name="vpl", bufs=1))
        vp = [vpl_pool.tile([128, H * (DK + 1)], f32r, tag=f"vp{j}",
                            name=f"vp{j}") for j in range(NJ)]

        dram = top.enter_context(tc.tile_pool(name="dram", bufs=1, space="DRAM"))
        ktp_in = dram.tile([D, DT], f32r, tag="ktp_in", name="ktp_in")
        ktp_out = dram.tile([NB * D, DT], f32r, tag="ktp_out", name="ktp_out")

        # ---- phase B: K^T projection (sharded) + AllGather ----
        with ExitStack() as ph:
            inp = ph.enter_context(tc.tile_pool(name="inp", bufs=1))
            wpool = ph.enter_context(tc.tile_pool(name="wpool", bufs=6))
            stg = ph.enter_context(tc.tile_pool(name="stg", bufs=4))
            psp = ph.enter_context(tc.tile_pool(name="psp", bufs=3, space="PSUM"))

            kin = [inp.tile([128, DT], f32r, tag=f"kin{i}", name=f"kin{i}")
                   for i in range(8)]
            for i in range(8):
                nc.sync.dma_start(kin[i][:], kT.ap()[i * 128:(i + 1) * 128, :])
            for s in range(8):
                pp = psp.tile([128, DT], f32, tag="pp", name="pp")
                for i in range(8):
                    w = wpool.tile([128, 128], f32r, tag="w", name="w")
                    nc.sync.dma_start(
                        w[:], wkT.ap()[i * 128:(i + 1) * 128, s * 128:(s + 1) * 128])
                    nc.tensor.matmul(pp[:], w[:], kin[i][:],
                                     start=(i == 0), stop=(i == 7))
                st = stg.tile([128, DT], f32r, tag="st", name="st")
                nc.scalar.add(st[:], pp[:], bk_sb[:, s:s + 1])
                nc.sync.dma_start(ktp_in[s * 128:(s + 1) * 128, :], st[:])

            nc.gpsimd.collective_compute(
                "AllGather", mybir.AluOpType.bypass, replica_groups=GROUPS,
                ins=[ktp_in.opt()], outs=[ktp_out.opt()])

            # Q^T projection (overlaps the collective)
            qin = [inp.tile([128, DT], f32r, tag=f"qin{i}", name=f"qin{i}")
                   for i in range(8)]
            for i in range(8):
                nc.sync.dma_start(qin[i][:], qT.ap()[i * 128:(i + 1) * 128, :])
            for s in range(8):
                pp = psp.tile([128, DT], f32, tag="pp", name="pp")
                for i in range(8):
                    w = wpool.tile([128, 128], f32r, tag="w", name="w")
                    nc.sync.dma_start(
                        w[:], wqT.ap()[i * 128:(i + 1) * 128, s * 128:(s + 1) * 128])
                    nc.tensor.matmul(pp[:], w[:], qin[i][:],
                                     start=(i == 0), stop=(i == 7))
                nc.scalar.add(qt_sb[s][:], pp[:], bq_sb[:, s:s + 1])

            # V projection, full batch (2048 keys), written straight into vp
            wvpool = ph.enter_context(tc.tile_pool(name="wvpool", bufs=1))
            wv = []
            for i in range(8):
                for dh in range(2):
                    wt = wvpool.tile([128, 512], f32r, tag=f"wv{i}_{dh}",
                                     name=f"wv{i}_{dh}")
                    nc.sync.dma_start(
                        wt[:], wvT.ap()[i * 128:(i + 1) * 128,
                                        dh * 512:(dh + 1) * 512])
                    wv.append((i, dh, wt))
            wv_tiles = {(i, dh): wt for i, dh, wt in wv}
            vinp = ph.enter_context(tc.tile_pool(name="vinp", bufs=24))
            for j in range(NJ):
                vin_j = []
                for i in range(8):
                    vt = vinp.tile([128, 128], f32r, tag="vin", name="vin")
                    nc.sync.dma_start(
                        vt[:], vT.ap()[i * 128:(i + 1) * 128,
                                       j * 128:(j + 1) * 128])
                    vin_j.append(vt)
                for dh in range(2):
                    pp = psp.tile([128, 512], f32, tag="ppv", name="ppv")
                    for i in range(8):
                        nc.tensor.matmul(pp[:], vin_j[i][:],
                                         wv_tiles[(i, dh)][:],
                                         start=(i == 0), stop=False)
                    nc.tensor.matmul(pp[:], ones_sb[0:1, :],
                                     bv_sb[0:1, dh * 512:(dh + 1) * 512],
                                     start=False, stop=True)
                    # evict into interleaved [V_h | 1] layout (heads 8dh..8dh+7)
                    dst = vp[j][:, 8 * dh * 65:(8 * dh + 8) * 65].rearrange(
                        "p (h d) -> p h d", h=8)[:, :, 0:DK]
                    with nc.allow_low_precision(reason="f32r V evict"):
                        nc.vector.tensor_copy(dst, pp[:].rearrange(
                            "p (h d) -> p h d", h=8))
                oc = vp[j][:].rearrange("p (h d) -> p h d", h=H)[:, :, DK:DK + 1]
                nc.sync.dma_start(
                    oc, onesin.ap().rearrange("p (h d) -> p h d", h=H)[:, :, 0:1])

        # ---- phase D: attention ----
        with ExitStack() as ph:
            ktf_pool = ph.enter_context(tc.tile_pool(name="ktf", bufs=8))
            ex_pool = ph.enter_context(tc.tile_pool(name="ex", bufs=3))
            rc_pool = ph.enter_context(tc.tile_pool(name="rc", bufs=2))
            bc_pool = ph.enter_context(tc.tile_pool(name="bc", bufs=2))
            ps_s = ph.enter_context(tc.tile_pool(name="ps_s", bufs=2, space="PSUM"))
            ps_ct = ph.enter_context(tc.tile_pool(name="ps_ct", bufs=2, space="PSUM"))
            ps_b = ph.enter_context(tc.tile_pool(name="ps_b", bufs=2, space="PSUM"))

            for t in range(8):                      # head pairs
                ktf = []
                for r in range(NB):
                    kt_t = ktf_pool.tile([128, DT], f32r, tag="ktf", name="ktf")
                    nc.sync.dma_start(
                        kt_t[:], ktp_out[r * D + t * 128: r * D + (t + 1) * 128, :])
                    ktf.append(kt_t)
                pct = [ps_ct.tile([65, DT], f32, tag="pct", name="pct")
                       for _ in range(2)]
                for j in range(NJ):
                    r, jl = j // NB, j % NB
                    pss = ps_s.tile([128, 2 * DT], f32, tag="pss", name="pss")
                    ex = ex_pool.tile([128, 2 * DT], f32r, tag="ex", name="ex")
                    # the two heads alternate array row-groups 0-63 / 64-127
                    for hh in range(2):
                        nc.tensor.matmul(
                            pss[:, hh * DT:(hh + 1) * DT],
                            ktf[r][hh * 64:hh * 64 + 64, jl * 128:(jl + 1) * 128],
                            qt_sb[t][hh * 64:hh * 64 + 64, :],
                            start=True, stop=True)
                    nc.scalar.activation(ex[:], pss[:], EXP, scale=0.125)
                    for hh in range(2):
                        h = 2 * t + hh
                        nc.tensor.matmul(
                            pct[hh][:], vp[j][:, h * 65:(h + 1) * 65],
                            ex[:, hh * DT:(hh + 1) * DT],
                            start=(j == 0), stop=(j == NJ - 1))
                for hh in range(2):
                    # NOTE: custom-DVE ops (reciprocal_approx_*) return garbage
                    # for partition-offset PSUM inputs; plain reciprocal works.
                    rc = rc_pool.tile([1, DT], f32r, tag="rc", name="rc")
                    with nc.allow_low_precision(reason="f32r softmax recip"):
                        nc.vector.reciprocal(rc[:], pct[hh][64:65, :])
                    rcr = rc[:]
                    pb = ps_b.tile([128, DT], f32, tag="pb", name="pb")
                    nc.tensor.matmul(pb[:], ones_sb[0:1, :], rcr,
                                     start=True, stop=True)
                    bcst = bc_pool.tile([128, DT], f32, tag="bc", name="bc")
                    nc.vector.tensor_copy(bcst[:], pb[:])
                    with nc.allow_low_precision(reason="f32r ctx normalize"):
                        nc.vector.tensor_mul(
                            ct_sb[t][hh * 64:hh * 64 + 64, :],
                            pct[hh][0:64, :], bcst[0:64, :])

        # ---- phase E: output projection ----
        with ExitStack() as ph:
            wo_pool = ph.enter_context(tc.tile_pool(name="wo", bufs=16))
            ob_pool = ph.enter_context(tc.tile_pool(name="ob", bufs=3))
            ps_o = ph.enter_context(tc.tile_pool(name="ps_o", bufs=3, space="PSUM"))
            for dh in range(2):
                wo = []
                for tt in range(8):
                    wt = wo_pool.tile([128, 512], f32r, tag="wo", name="wo")
                    nc.sync.dma_start(
                        wt[:], woT.ap()[tt * 128:(tt + 1) * 128,
                                        dh * 512:(dh + 1) * 512])
                    wo.append(wt)
                for qs in range(4):
                    po = ps_o.tile([128, 512], f32, tag="po", name="po")
                    for tt in range(8):
                        nc.tensor.matmul(po[:],
                                         ct_sb[tt][:, qs * 128:(qs + 1) * 128],
                                         wo[tt][:], start=(tt == 0), stop=False)
                    nc.tensor.matmul(po[:], ones_sb[0:1, :],
                                     bo_sb[0:1, dh * 512:(dh + 1) * 512],
                                     start=False, stop=True)
                    ob = ob_pool.tile([128, 512], f32, tag="ob", name="ob")
                    nc.scalar.copy(ob[:], po[:])
                    nc.sync.dma_start(
                        out.ap()[qs * 128:(qs + 1) * 128,
                                 dh * 512:(dh + 1) * 512], ob[:])

    nc.compile()
    return nc


def _get_nc():
    if "nc" not in _CACHE:
        _CACHE["nc"] = _build()
    return _CACHE["nc"]


def _prep_in_maps(q, k, v, Wq, bq, Wk, bk, Wv, bv, Wo, bo):
    f = lambda a: np.ascontiguousarray(np.asarray(a, dtype=np.float32))
    qT = f(np.asarray(q, dtype=np.float32).reshape(B * S, D).T)
    kT = f(np.asarray(k, dtype=np.float32).reshape(B * S, D).T)
    vT = f(np.asarray(v, dtype=np.float32).reshape(B * S, D).T)
    shared = {
        "wqT": f(np.asarray(Wq).T), "wkT": f(np.asarray(Wk).T),
        "wvT": f(np.asarray(Wv).T), "woT": f(np.asarray(Wo).T),
        "bq": f(bq), "bk": f(bk), "bv": f(bv), "bo": f(bo),
        "onesin": np.ones((128, 128), np.float32),
    }
    in_maps = []
    for c in range(N_CORES):
        g = c // NB
        in_maps.append({
            "qT": np.ascontiguousarray(qT[:, c * DT:(c + 1) * DT]),
            "kT": np.ascontiguousarray(kT[:, c * DT:(c + 1) * DT]),
            "vT": np.ascontiguousarray(vT[:, g * KB:(g + 1) * KB]),
            **shared,
        })
    return in_maps


def _run(in_maps, trace=False, **kw):
    from concourse.bass_utils import run_bass_kernel_spmd
    nc = _get_nc()
    res = run_bass_kernel_spmd(nc, in_maps, core_ids=list(range(N_CORES)),
                               trace=trace, **kw)
    full = np.concatenate([res.results[c]["out"] for c in range(N_CORES)],
                          axis=0).reshape(B, S, D)
    return full, res


def kernel(q, k, v, Wq, bq, Wk, bk, Wv, bv, Wo, bo):
    in_maps = _prep_in_maps(q, k, v, Wq, bq, Wk, bk, Wv, bv, Wo, bo)
    full, _ = _run(in_maps, trace=False)
    return full
